# revision 16
# baseline (speedup 1.0000x reference)
"""PointPillarScatter (intersweep, 3 bins) Trainium2 Bass kernel. (v2)"""

import numpy as np
import ml_dtypes

import concourse.bass as bass
import concourse.tile as tile
from concourse import bacc, mybir
from concourse.bass_utils import run_bass_kernel_spmd

B = 4
C = 64
NX = 432
NY = 496
NBINS = 3
NCORES = 8

NQ = NBINS * B * 4          # 48 quarter-canvases
YQ = NY // 4                # 124 y-rows per quarter
QCELLS = YQ * NX            # 53568 cells per quarter
QPC = NQ // NCORES          # 6 quarters per core
PAIRS = QPC // 2            # 3 pairs per core
NW = 496                    # cells per window (<=512: one PSUM bank)
WPP = QCELLS // NW          # 108 windows per pair
WINDOWS = PAIRS * WPP       # 324 windows per core
CH = 27                     # windows per staging chunk / out-DMA
NCHUNKS = WPP // CH         # 4 chunks per pair
RP = 64                     # pillar slots per window per half (max seen 41)
PSW = 4                     # windows per PSUM tile (4 banks)
NPT = (CH + PSW - 1) // PSW  # psum tiles per chunk (6x4 + 1x3)

_cache = {}


def _build():
    nc = bacc.Bacc(trn_type="TRN2")
    fp16 = mybir.dt.float16
    f32 = mybir.dt.float32
    lhst_d = nc.dram_tensor("lhst", [2 * RP, WINDOWS, C], fp16,
                            kind="ExternalInput")
    iota_d = nc.dram_tensor("iotat", [2 * RP, NW], fp16, kind="ExternalInput")
    relc_d = nc.dram_tensor("relc", [2 * RP, WINDOWS], f32,
                            kind="ExternalInput")
    out_d = nc.dram_tensor("out", [PAIRS, NCHUNKS, 128, CH, NW], fp16,
                           kind="ExternalOutput")

    with tile.TileContext(nc) as tc:
        with (
            tc.tile_pool(name="const", bufs=1) as constp,
            tc.tile_pool(name="ltp", bufs=1) as ltp,
            tc.tile_pool(name="maskp", bufs=10) as maskp,
            tc.tile_pool(name="stage", bufs=3) as stagep,
            tc.tile_pool(name="psum", bufs=2, space=bass.MemorySpace.PSUM) as psump,
        ):
            iota = constp.tile([128, NW], fp16, name="iota")
            relc = constp.tile([128, WINDOWS], f32, name="relc")
            nc.scalar.dma_start(out=iota[:], in_=iota_d[:])
            nc.scalar.dma_start(out=relc[:], in_=relc_d[:])
            lts = [ltp.tile([128, CH, 128], fp16, name=f"lt{pp}",
                            tag=f"lt{pp}") for pp in range(2)]
            for pp in range(2):
                nc.vector.memset(lts[pp][:], 0.0)
            for pair in range(PAIRS):
                for ch in range(NCHUNKS):
                    g0 = pair * WPP + ch * CH
                    pp = (pair * NCHUNKS + ch) % 2
                    lt = lts[pp]
                    nc.scalar.dma_start(out=lt[0:RP, :, 0:C],
                                        in_=lhst_d[0:RP, g0:g0 + CH, :])
                    nc.scalar.dma_start(out=lt[RP:2 * RP, :, C:128],
                                        in_=lhst_d[RP:2 * RP, g0:g0 + CH, :])
                    st = stagep.tile([128, CH, NW], fp16, name="st")
                    for t in range(NPT):
                        nw = min(PSW, CH - PSW * t)
                        pt = psump.tile([128, PSW, 512], f32, name="pt")
                        for j in range(nw):
                            w = PSW * t + j
                            mask = maskp.tile([128, NW], fp16, name="mask")
                            nc.vector.tensor_scalar(
                                out=mask[:],
                                in0=iota[:],
                                scalar1=relc[:, g0 + w:g0 + w + 1],
                                scalar2=None,
                                op0=mybir.AluOpType.is_equal,
                            )
                            nc.tensor.matmul(pt[:, j, 0:NW], lt[:, w, :],
                                             mask[:], start=True, stop=True)
                        dst = st[:, PSW * t:PSW * t + nw, :]
                        src = pt[:, 0:nw, 0:NW]
                        ci = pair * NCHUNKS + ch
                        if t == 3 or (t == 6 and ci % 2 == 0):
                            nc.vector.tensor_copy(out=dst, in_=src)
                        else:
                            nc.scalar.copy(out=dst, in_=src)
                    nc.sync.dma_start(out=out_d[pair, ch], in_=st[:])
    nc.compile()
    return nc


def _pack(inputs):
    lhst = np.zeros((NCORES, 2 * RP, WINDOWS, C), np.float16)
    iota = np.broadcast_to(np.arange(NW, dtype=np.float32),
                           (NCORES, 2 * RP, NW)).astype(np.float16)
    relc_a = np.full((NCORES, 2 * RP, WINDOWS), -1.0, np.float32)

    for bin_i in range(NBINS):
        feats = np.asarray(inputs[f"pillar_features_bin_{bin_i}"],
                           np.float32).astype(np.float16)
        coords = np.asarray(inputs[f"voxel_coords_bin_{bin_i}"])
        cb = np.asarray(coords[:, 0], np.int64)
        cy = np.asarray(coords[:, 2], np.int64)
        cx = np.asarray(coords[:, 3], np.int64)
        for b in range(B):
            rows_b = np.nonzero(cb == b)[0]
            y_b, x_b = cy[rows_b], cx[rows_b]
            for yq in range(4):
                q = bin_i * 16 + b * 4 + yq
                core, j = divmod(q, QPC)
                pair, half = divmod(j, 2)
                sel = (y_b >= YQ * yq) & (y_b < YQ * (yq + 1))
                rows = rows_b[sel]
                qcell = (y_b[sel] - YQ * yq) * NX + x_b[sel]
                w = qcell // NW
                rel = qcell % NW
                order = np.argsort(w, kind="stable")
                rows, w, rel = rows[order], w[order], rel[order]
                cnt = np.bincount(w, minlength=WPP)
                if cnt.max() > RP:
                    raise OverflowError(int(cnt.max()))
                off = np.concatenate([[0], np.cumsum(cnt)[:-1]])
                slot = np.arange(len(rows)) - off[w]
                r = half * RP + slot
                wins = pair * WPP + w
                lhst[core, r, wins, :] = feats[rows]
                relc_a[core, r, wins] = rel
    return [{"lhst": lhst[c], "iotat": iota[c], "relc": relc_a[c]}
            for c in range(NCORES)]


def _run(inputs, trace=False):
    if "nc" not in _cache:
        _cache["nc"] = _build()
    nc = _cache["nc"]
    in_maps = _pack(inputs)
    res = run_bass_kernel_spmd(nc, in_maps, core_ids=list(range(NCORES)),
                               trace=trace)
    outs = [np.zeros((B, C, NY, NX), np.float32) for _ in range(NBINS)]
    for core in range(NCORES):
        blk = np.asarray(res.results[core]["out"])
        for pair in range(PAIRS):
            a = blk[pair].transpose(1, 0, 2, 3).reshape(128, QCELLS)
            for half in range(2):
                q = core * QPC + pair * 2 + half
                bin_i, rem = divmod(q, 16)
                b, yq = divmod(rem, 4)
                outs[bin_i][b, :, YQ * yq:YQ * (yq + 1), :] = (
                    a[half * C:(half + 1) * C]
                    .reshape(C, YQ, NX).astype(np.float32))
    return tuple(outs), res


def kernel(**inputs):
    out, _ = _run(inputs)
    return out


def kernel_traced(**inputs):
    """Like kernel() but also returns BassKernelResults (for test.py)."""
    return _run(inputs, trace=True)
